# revision 3
# baseline (speedup 1.0000x reference)
"""Trainium2 Bass kernel for causal local-window self-attention — v2.

v1 replicated all weights to every core (176MB host->device per call).
v2 minimizes per-call host work + transfer:
  - x uploaded token-major, sequence-sharded WITHOUT transform: per-core
    slices are views, so the runner's concat is the only host copy.
  - w_attn / w_proj uploaded ROW-sharded (per-core slices are views),
    packed into one [128, 4096] staging tensor on device and
    AllGathered over NeuronLink to reconstruct the full weights in each
    core's DRAM (14MB over links instead of 112MB over the host link).
  - a 256-token halo input per core (the only strided host copy, 8MB).
  - output is token-major [512, 1024] per core, so the full output is
    just the concatenation — no host transpose.
  - q-scale (1/sqrt(hd)) folded into the on-device PSUM->SBUF copy.

Compute per core (identical SPMD program), all matmuls float32r:
  x^T built from token-major x by PE-transpose with an identity matrix;
  q^T,k^T feature-major; v token-major packed as V_aug[k,65] with a ones
  column so the AV matmul also yields softmax denominators; scores
  s^T=[keys,q] + band mask on DVE; exp on ACT (no max subtraction:
  logits are O(5), fp32-safe); denominators inverted on DVE and
  partition-broadcast with a selector matmul; out = (y^T)^T @ w_proj
  computed token-major by using y^T chunks as stationary operands.

Shapes (hardcoded): B=2, T=2048, C=1024, H=16, hd=64, window=256.
"""

import numpy as np

import concourse.bass as bass
import concourse.mybir as mybir
from concourse.tile import TileContext
from concourse.bass_utils import run_bass_kernel_spmd

F32 = mybir.dt.float32
F32R = mybir.dt.float32r

N_CORES = 8
B, T, C = 2, 2048, 1024
H, HD, W = 16, 64, 256
T_OWN = 512          # queries per core
HALO = 256
T_LOC = T_OWN + HALO  # keys/values per core
NEG = -1e9
QSCALE = 1.0 / np.sqrt(HD)


# ---------------------------------------------------------------------------
# BIR post-pass: this walrus build only accepts one sync-wait per CTRL-class
# instruction; hoist extra waits onto NoOps inserted just before.
# ---------------------------------------------------------------------------
def _split_excess_waits(nc, max_waits=1):
    for fn in nc.m.functions:
        for blk in fn.blocks:
            insts = blk.instructions
            i = 0
            while i < len(insts):
                inst = insts[i]
                si = inst.sync_info
                if si is not None and si.on_wait and len(si.on_wait) > max_waits:
                    waits = list(si.on_wait)
                    keep = waits[-max_waits:]
                    extra = waits[:-max_waits]
                    nops = []
                    for j in range(0, len(extra), max_waits):
                        nop = mybir.InstNoOp(
                            name=nc.get_next_instruction_name(),
                            sync_info=mybir.SyncInfo(
                                on_wait=extra[j : j + max_waits], on_update=[]
                            ),
                            bass_nofuse=True,
                            engine=inst.engine,
                        )
                        nops.append(nop)
                    inst.sync_info = mybir.SyncInfo(
                        on_wait=keep, on_update=list(si.on_update)
                    )
                    for k, nop in enumerate(nops):
                        insts.insert(i + k, nop)
                        nc.register_instruction(nop)
                    i += len(nops)
                i += 1
    return nc


# ---------------------------------------------------------------------------
# Device program (identical on all 8 cores)
# ---------------------------------------------------------------------------
def build_nc(debug=False, reps=None):
    nc = bass.Bass(num_devices=N_CORES)

    xo = nc.dram_tensor("xo", [T_OWN, C], F32R, kind="ExternalInput")
    xh = nc.dram_tensor("xh", [HALO, C], F32R, kind="ExternalInput")
    wac = nc.dram_tensor("wac", [128, 3 * C], F32R, kind="ExternalInput")
    wpc = nc.dram_tensor("wpc", [128, C], F32R, kind="ExternalInput")
    # flag: 1.0 on batch-start cores (halo keys invalid), else 0.0
    flag = nc.dram_tensor("flag", [128, 1], F32, kind="ExternalInput")
    out = nc.dram_tensor("out", [T_OWN, C], F32, kind="ExternalOutput")

    wstage = nc.dram_tensor("wstage", [128, 4 * C], F32R)
    wg = nc.dram_tensor("wg", [C, 4 * C], F32R, addr_space="Shared")
    den_dram = nc.dram_tensor("den_dram", [16, T_OWN], F32)

    # column bases inside wg
    WQ, WK, WV, WP = 0, C, 2 * C, 3 * C

    with TileContext(nc) as tc:
        with (
            tc.tile_pool(name="big", bufs=1) as big,
            tc.tile_pool(name="xtp", bufs=2) as xtp,
            tc.tile_pool(name="wtiles", bufs=2) as wtiles,
            tc.tile_pool(name="wvtiles", bufs=1) as wvtiles,
            tc.tile_pool(name="pt", bufs=2) as ptpool,
            tc.tile_pool(name="stage", bufs=2) as stage,
            tc.tile_pool(name="psq", bufs=2, space="PSUM") as psq,
            tc.tile_pool(name="pss", bufs=3, space="PSUM") as pss_pool,
            tc.tile_pool(name="psy", bufs=2, space="PSUM") as psy_pool,
        ):
          for _rep in range(reps or 1):
              # ---- stage + AllGather weights ---------------------------------
              nc.sync.dma_start(out=wstage[:, : 3 * C], in_=wac[:])
              nc.sync.dma_start(out=wstage[:, 3 * C :], in_=wpc[:])
              nc.gpsimd.collective_compute(
                  "AllGather",
                  mybir.AluOpType.bypass,
                  replica_groups=[list(range(N_CORES))],
                  ins=[wstage[:].opt()],
                  outs=[wg[:].opt()],
              )

              # ---- constants generated on device ----------------------------
              # band mask mk[r(part), qb, j, col]: 0 where query col of
              # q-block qb may attend key d=j*128+r, else NEG:
              #   valid = (col >= d-256) & (col < d)
              # batch-start cores additionally need d+qb*256 >= 256 (halo
              # invalid); that term is scaled by the per-core flag input.
              mk = big.tile([128, 2, 4, 256], F32, tag="mk")
              nc.vector.memset(mk[:], 0.0)
              nc.vector.affine_select(
                  mk[:], mk[:], [[0, 2], [-128, 4], [1, 256]],
                  mybir.AluOpType.is_ge, NEG, base=256, channel_multiplier=-1,
              )
              nc.vector.affine_select(
                  mk[:], mk[:], [[0, 2], [128, 4], [-1, 256]],
                  mybir.AluOpType.is_ge, NEG, base=-1, channel_multiplier=1,
              )
              extra = big.tile([128, 2, 4, 256], F32, tag="extra")
              nc.vector.memset(extra[:], 0.0)
              nc.vector.affine_select(
                  extra[:], extra[:], [[256, 2], [128, 4], [0, 256]],
                  mybir.AluOpType.is_ge, NEG, base=-256, channel_multiplier=1,
              )
              flag_sb = big.tile([128, 1], F32, tag="flag")
              nc.sync.dma_start(out=flag_sb[:], in_=flag[:])
              nc.vector.tensor_mul(
                  out=extra[:], in0=extra[:],
                  in1=flag_sb[:, None, None, :].to_broadcast((128, 2, 4, 256)),
              )
              nc.vector.tensor_add(out=mk[:], in0=mk[:], in1=extra[:])

              # sel[h, col] = 1 iff col in [64h, 64h+64): recip broadcast map
              self_f = big.tile([16, C], F32, tag="selF")
              nc.vector.memset(self_f[:], 1.0)
              nc.vector.affine_select(
                  self_f[:], self_f[:], [[1, C]],
                  mybir.AluOpType.is_ge, 0.0, base=0, channel_multiplier=-64,
              )
              nc.vector.affine_select(
                  self_f[:], self_f[:], [[-1, C]],
                  mybir.AluOpType.is_ge, 0.0, base=63, channel_multiplier=64,
              )
              sel_sb = big.tile([16, C], F32R, tag="sel")
              nc.vector.tensor_copy(out=sel_sb[:], in_=self_f[:])

              # identity for PE transposes
              id_f = big.tile([128, 128], F32, tag="idF")
              nc.vector.memset(id_f[:], 1.0)
              nc.vector.affine_select(
                  id_f[:], id_f[:], [[-1, 128]],
                  mybir.AluOpType.is_equal, 0.0, base=0, channel_multiplier=1,
              )
              id_sb = big.tile([128, 128], F32R, tag="ident")
              nc.vector.tensor_copy(out=id_sb[:], in_=id_f[:])

              ones_sb = big.tile([128, 1], F32, tag="ones")
              nc.vector.memset(ones_sb[:], 1.0)

              # ---- x^T via PE transpose -------------------------------------
              # local token order: [0,256) = halo, [256,768) = own
              xts = big.tile([128, 8, T_LOC], F32R, tag="xts")
              for t in range(6):
                  xt = xtp.tile([128, C], F32R, tag="xt")
                  if t < 2:
                      nc.sync.dma_start(out=xt[:], in_=xh[t * 128 : (t + 1) * 128, :])
                  else:
                      nc.sync.dma_start(
                          out=xt[:], in_=xo[(t - 2) * 128 : (t - 1) * 128, :]
                      )
                  for g in range(2):
                      ps = psq.tile([128, 512], F32, tag="ps_qkv")
                      for f4 in range(4):
                          f = g * 4 + f4
                          nc.tensor.matmul(
                              ps[:, f4 * 128 : (f4 + 1) * 128],
                              xt[:, f * 128 : (f + 1) * 128],
                              id_sb[:],
                              start=True,
                              stop=True,
                          )
                      nc.scalar.copy(
                          out=xts[:, g * 4 : (g + 1) * 4, t * 128 : (t + 1) * 128],
                          in_=ps[:].rearrange("p (f m) -> p f m", m=128),
                      )

              qTs = big.tile([128, 8, T_OWN], F32R, tag="qTs")
              kTs = big.tile([128, 8, T_LOC], F32R, tag="kTs")
              # V_aug: [part(keys%128), kc, head, 65]; col 64 of each head is 1.0
              vaug = big.tile([128, 6, 16, 65], F32R, tag="vaug")
              yTs = big.tile([128, 8, T_OWN], F32R, tag="yTs")
              recips = big.tile([16, T_OWN], F32, tag="recips")
              recips_r = big.tile([16, T_OWN], F32R, tag="recips_r")

              # ---- q^T (scaled), k^T (feature-major) ------------------------
              for oc in range(8):
                  wsl = wtiles.tile([128, 8, 128], F32R, tag="wsl")
                  nc.sync.dma_start(
                      out=wsl[:],
                      in_=wg[:, WQ + oc * 128 : WQ + (oc + 1) * 128].rearrange(
                          "(i p) m -> p i m", p=128
                      ),
                  )
                  ps = psq.tile([128, 512], F32, tag="ps_qkv")
                  for ic in range(8):
                      nc.tensor.matmul(
                          ps[:], wsl[:, ic], xts[:, ic, HALO:], start=(ic == 0), stop=(ic == 7)
                      )
                  nc.scalar.mul(qTs[:, oc], ps[:], QSCALE)
              for oc in range(8):
                  wsl = wtiles.tile([128, 8, 128], F32R, tag="wsl")
                  nc.sync.dma_start(
                      out=wsl[:],
                      in_=wg[:, WK + oc * 128 : WK + (oc + 1) * 128].rearrange(
                          "(i p) m -> p i m", p=128
                      ),
                  )
                  for hf in range(2):
                      ps = psq.tile([128, 512], F32, tag="ps_qkv")
                      for ic in range(8):
                          nc.tensor.matmul(
                              ps[:, :384],
                              wsl[:, ic],
                              xts[:, ic, hf * 384 : (hf + 1) * 384],
                              start=(ic == 0),
                              stop=(ic == 7),
                          )
                      nc.scalar.copy(out=kTs[:, oc, hf * 384 : (hf + 1) * 384], in_=ps[:, :384])

              # ---- v (token-major) + ones column ----------------------------
              for h2 in range(2):
                  wvsl = wvtiles.tile([128, 8, 512], F32R, tag="wvsl")
                  nc.sync.dma_start(
                      out=wvsl[:],
                      in_=wg[:, WV + h2 * 512 : WV + (h2 + 1) * 512].rearrange(
                          "(i p) m -> p i m", p=128
                      ),
                  )
                  for kc in range(6):
                      ps = psq.tile([128, 512], F32, tag="ps_qkv")
                      for ic in range(8):
                          nc.tensor.matmul(
                              ps[:],
                              xts[:, ic, kc * 128 : (kc + 1) * 128],
                              wvsl[:, ic],
                              start=(ic == 0),
                              stop=(ic == 7),
                          )
                      nc.scalar.copy(
                          out=vaug[:, kc, h2 * 8 : (h2 + 1) * 8, 0:64],
                          in_=ps[:].rearrange("p (h d) -> p h d", d=64),
                      )
              for kc in range(6):
                  nc.vector.tensor_copy(
                      out=vaug[:, kc, :, 64:65],
                      in_=ones_sb[:, None, :].to_broadcast((128, 16, 1)),
                  )

              # ---- attention: per head, q-blocks of 256, key chunks of 128 --
              for h in range(16):
                  pb = (h % 2) * 64  # partition base of this head's features
                  oc = h // 2
                  for qb in range(2):
                      ptile = ptpool.tile([128, 4, 256], F32R, tag="pt")
                      for j in range(4):
                          ps = pss_pool.tile([128, 256], F32, tag="ps_s")
                          nc.tensor.matmul(
                              ps[:],
                              kTs[pb : pb + 64, oc, (qb * 2 + j) * 128 : (qb * 2 + j + 1) * 128],
                              qTs[pb : pb + 64, oc, qb * 256 : (qb + 1) * 256],
                              start=True,
                              stop=True,
                          )
                          nc.vector.tensor_add(out=ps[:], in0=ps[:], in1=mk[:, qb, j])
                          nc.scalar.activation(
                              out=ptile[:, j], in_=ps[:], func=mybir.ActivationFunctionType.Exp
                          )
                      ya = psy_pool.tile([128, 256], F32, tag="ps_y")
                      for j in range(4):
                          nc.tensor.matmul(
                              ya[:65],
                              vaug[:, qb * 2 + j, h],
                              ptile[:, j],
                              start=(j == 0),
                              stop=(j == 3),
                          )
                      # stash denominator row; normalize y^T after recip bcast
                      db = stage.tile([1, 256], F32, tag="den")
                      nc.vector.tensor_copy(out=db[:], in_=ya[64:65])
                      nc.sync.dma_start(
                          out=den_dram[h : h + 1, qb * 256 : (qb + 1) * 256],
                          in_=db[0:1, :],
                      )
                      # keep unnormalized y^T in SBUF for now
                      nc.vector.tensor_copy(
                          out=yTs[pb : pb + 64, oc, qb * 256 : (qb + 1) * 256], in_=ya[0:64]
                      )

              # ---- reciprocal + partition-broadcast + normalize -------------
              nc.sync.dma_start(out=recips[:], in_=den_dram[:])
              nc.vector.reciprocal(out=recips[:], in_=recips[:])
              nc.vector.tensor_copy(out=recips_r[:], in_=recips[:])
              for t in range(8):
                  rb = psq.tile([128, 512], F32, tag="ps_qkv")
                  nc.tensor.matmul(
                      rb[:], sel_sb[:, t * 128 : (t + 1) * 128], recips_r[:], start=True, stop=True
                  )
                  rb_sb = stage.tile([128, 512], F32, tag="rb_sb")
                  nc.scalar.copy(out=rb_sb[:], in_=rb[:])
                  for i in range(2):  # the two heads of the pair
                      h = 2 * t + i
                      pb = (h % 2) * 64
                      nc.vector.tensor_mul(
                          out=yTs[pb : pb + 64, t],
                          in0=yTs[pb : pb + 64, t],
                          in1=rb_sb[pb : pb + 64, :],
                      )

              # ---- out projection, token-major: out = (y^T)^T @ w_proj ------
              for half in range(2):
                  wph = wvtiles.tile([128, 8, 512], F32R, tag="wvsl")
                  nc.sync.dma_start(
                      out=wph[:],
                      in_=wg[:, WP + half * 512 : WP + (half + 1) * 512].rearrange(
                          "(i p) m -> p i m", p=128
                      ),
                  )
                  for tb in range(4):
                      ps = psq.tile([128, 512], F32, tag="ps_qkv")
                      for ic in range(8):
                          nc.tensor.matmul(
                              ps[:],
                              yTs[:, ic, tb * 128 : (tb + 1) * 128],
                              wph[:, ic],
                              start=(ic == 0),
                              stop=(ic == 7),
                          )
                      ot = stage.tile([128, 512], F32, tag="ot")
                      nc.scalar.copy(out=ot[:], in_=ps[:])
                      nc.sync.dma_start(
                          out=out[tb * 128 : (tb + 1) * 128, half * 512 : (half + 1) * 512],
                          in_=ot[:],
                      )

    _split_excess_waits(nc)
    return nc


# ---------------------------------------------------------------------------
# Host-side sharding / unsharding
# ---------------------------------------------------------------------------
_FLAG1 = np.ones((128, 1), np.float32)
_FLAG0 = np.zeros((128, 1), np.float32)


def make_in_maps(x, w_attn, w_proj):
    xf = np.asarray(x, dtype=np.float32).reshape(B * T, C)
    wa = np.asarray(w_attn, dtype=np.float32)
    wp = np.asarray(w_proj, dtype=np.float32)

    in_maps = []
    for c in range(N_CORES):
        start = c * T_OWN
        # halo rows: previous 256 tokens; for batch-start chunks the mask
        # invalidates them, any real rows do (use the wrap-around slice).
        hs = (start - HALO) % (B * T)
        in_maps.append(
            {
                "xo": xf[start : start + T_OWN],
                "xh": xf[hs : hs + HALO],
                "wac": wa[c * 128 : (c + 1) * 128],
                "wpc": wp[c * 128 : (c + 1) * 128],
                "flag": _FLAG1 if c % 4 == 0 else _FLAG0,
            }
        )
    return in_maps


def gather_output(results):
    out = np.concatenate([results[c]["out"] for c in range(N_CORES)], axis=0)
    return out.reshape(B, T, C)


_CACHED = {}
_FLAGS_GLOBAL = np.concatenate(
    [_FLAG1 if c % 4 == 0 else _FLAG0 for c in range(N_CORES)], axis=0
)


class _AxonRunner:
    """Persistent-executable SPMD runner for the axon/PJRT path.

    vs run_bass_kernel_spmd per call: keeps the jitted executable and the
    output scratch buffers alive across calls, and takes the already-
    concatenated global arrays (x and the weights shard back into exactly
    the original arrays, so no per-core concat copies are needed).
    """

    def __init__(self, nc):
        import jax
        from jax.sharding import Mesh, PartitionSpec, NamedSharding
        from jax.experimental.shard_map import shard_map
        from concourse import bass2jax

        bass2jax.install_neuronx_cc_hook()
        part_name = nc.partition_id_tensor.name if nc.partition_id_tensor else None
        in_names, out_names, out_avals = [], [], []
        for alloc in nc.m.functions[0].allocations:
            if not isinstance(alloc, mybir.MemoryLocationSet):
                continue
            name = alloc.memorylocations[0].name
            if alloc.kind == "ExternalInput":
                if name != part_name:
                    in_names.append(name)
            elif alloc.kind == "ExternalOutput":
                out_names.append(name)
                out_avals.append(
                    jax.core.ShapedArray(
                        tuple(alloc.tensor_shape), mybir.dt.np(alloc.dtype)
                    )
                )
        all_names = in_names + out_names
        if part_name is not None:
            all_names = all_names + [part_name]

        def _body(*args):
            operands = list(args)
            if part_name is not None:
                operands.append(bass2jax.partition_id_tensor())
            return tuple(
                bass2jax._bass_exec_p.bind(
                    *operands,
                    out_avals=tuple(out_avals),
                    in_names=tuple(all_names),
                    out_names=tuple(out_names),
                    lowering_input_output_aliases=(),
                    sim_require_finite=True,
                    sim_require_nnan=True,
                    nc=nc,
                )
            )

        devices = jax.devices()[:N_CORES]
        mesh = Mesh(np.asarray(devices), ("core",))
        spec = PartitionSpec("core")
        n_args = len(in_names) + len(out_names)
        self._fn = jax.jit(
            shard_map(
                _body,
                mesh=mesh,
                in_specs=(spec,) * n_args,
                out_specs=(spec,) * len(out_names),
                check_rep=False,
            ),
            keep_unused=True,
        )
        self._sh = NamedSharding(mesh, spec)
        # output scratch, created on device once and reused (not donated)
        self._scratch = [
            jax.device_put(
                np.zeros((N_CORES * a.shape[0], *a.shape[1:]), a.dtype), self._sh
            )
            for a in out_avals
        ]
        self._in_names = in_names
        self._jax = jax

    def run(self, globals_by_name):
        dev = [
            self._jax.device_put(globals_by_name[n], self._sh)
            for n in self._in_names
        ]
        outs = self._fn(*dev, *self._scratch)
        return np.asarray(outs[0])  # single output: token-major [B*T, C]


def kernel(x, w_attn, w_proj):
    if "nc" not in _CACHED:
        _CACHED["nc"] = build_nc()
    from concourse.bass_utils import axon_active

    if not axon_active():
        in_maps = make_in_maps(x, w_attn, w_proj)
        res = run_bass_kernel_spmd(_CACHED["nc"], in_maps, list(range(N_CORES)))
        return gather_output(res.results)

    if "runner" not in _CACHED:
        _CACHED["runner"] = _AxonRunner(_CACHED["nc"])
    xf = np.ascontiguousarray(np.asarray(x, dtype=np.float32).reshape(B * T, C))
    wa = np.ascontiguousarray(np.asarray(w_attn, dtype=np.float32))
    wp = np.ascontiguousarray(np.asarray(w_proj, dtype=np.float32))
    halo = np.concatenate(
        [xf[(c * T_OWN - HALO) % (B * T) : (c * T_OWN - HALO) % (B * T) + HALO]
         for c in range(N_CORES)],
        axis=0,
    )
    out = _CACHED["runner"].run(
        {"xo": xf, "xh": halo, "wac": wa, "wpc": wp, "flag": _FLAGS_GLOBAL}
    )
    return out.reshape(B, T, C)


if __name__ == "__main__":
    rng = np.random.default_rng(0)
    x = rng.standard_normal((B, T, C)).astype(np.float32)
    wa = (rng.standard_normal((C, 3 * C)) / np.sqrt(C)).astype(np.float32)
    wpj = (rng.standard_normal((C, C)) / np.sqrt(C)).astype(np.float32)
    out = kernel(x, wa, wpj)
    print("out", out.shape, out.dtype, np.abs(out).max())
